# revision 35
# baseline (speedup 1.0000x reference)
"""Binarized 3-layer MLP (B=8192, H=4096) on 8 Trainium2 NeuronCores.

Strategy: data-parallel over batch (1024 rows/core), weights replicated.
All matmul operands are exactly +-1, so the GEMMs are exact in bf16/fp8
(products +-1, fp32 PSUM accumulation of <=4096 terms). BatchNorm+binarize
folds into an integer threshold per output channel: the GEMM output y is an
even integer in [-4096, 4096] and gamma*rsqrt(var+eps) > 0, so
  sign(BN(y)) = +1  <=>  y >= T_o
for an even-integer threshold T_o computed on the host. On-device this is a
single ScalarE Sign activation with per-partition bias 1 - T_o (y + 1 - T_o
is an odd integer, so no 0-boundary ambiguity).

Layout is feature-major throughout: activations live in SBUF as
[128 partitions (h within chunk), 32 chunks x 1024 batch]; each layer is
32 o-tiles x (32 h-chunks x 2 batch-halves) accumulating matmuls
(lhsT = W^T tile [128h, 128o], rhs = act [128h, 512b]) followed by one
Sign over the [128, 1024] PSUM tile, written directly into the other
activation plane. No transposes, no collectives.
"""

import numpy as np
import ml_dtypes

N_CORES = 8
B, H, L, NCOUT = 8192, 4096, 3, 10
BC = B // N_CORES          # batch per core
NT = H // 128              # 32 tiles of 128 along any H axis
BN_EPS = np.float32(1e-5)
TN_EPS = np.float32(1e-4)
HALF = BC // 2             # 512: one PSUM bank of fp32 per matmul

TRACE = False              # test harness may flip this for NTFF profiling
TRACE_DIR = None
LAST_EXEC_NS = None
MODE = "fp8dr"             # "fp8dr" (fp8 + DoubleRow, 2x contraction/MM) or "bf16"
ND = H // 256              # 16 double-row chunks of 256 along contraction

_BUILD_CACHE = {}


def _split_multi_waits(nc):
    """walrus' CoreV3 codegen rejects instructions carrying more than one
    semaphore wait. Hoist all-but-one wait of any multi-wait instruction
    into standalone NoOps (same engine, placed immediately before)."""
    import bass_rust
    import concourse.mybir as mybir

    n = 0
    for f in nc.m.functions:
        for blk in f.blocks:
            out = []
            changed = False
            for inst in blk.instructions:
                si = inst.sync_info
                if si is not None and si.on_wait and len(si.on_wait) > 1:
                    waits = list(si.on_wait)
                    for w in waits[:-1]:
                        n += 1
                        nop = mybir.InstNoOp(name=f"waitsplit_{n}", ins=[], outs=[])
                        nop.engine = inst.engine
                        nop.sync_info = bass_rust.SyncInfo(on_wait=[w], on_update=[])
                        out.append(nop)
                    inst.sync_info = bass_rust.SyncInfo(
                        on_wait=[waits[-1]], on_update=list(si.on_update or [])
                    )
                    changed = True
                out.append(inst)
            if changed:
                blk.instructions = out
    return nc


def _build(mode):
    if mode in _BUILD_CACHE:
        return _BUILD_CACHE[mode]

    import concourse.bass as bass
    import concourse.mybir as mybir
    from concourse.tile import TileContext

    dt_w = mybir.dt.float8e4 if mode == "fp8dr" else mybir.dt.bfloat16
    f32 = mybir.dt.float32

    wout_w = NT * NCOUT
    nc = bass.Bass()
    xin = nc.dram_tensor("x", [ND, 128, 2 * BC], dt_w, kind="ExternalInput")
    win = nc.dram_tensor("w", [L, NT, 128, H], dt_w, kind="ExternalInput")
    biasin = nc.dram_tensor("bias", [128, L * NT], f32, kind="ExternalInput")
    woutin = nc.dram_tensor("wout", [128, wout_w], dt_w, kind="ExternalInput")
    outd = nc.dram_tensor("out", [NCOUT, BC], f32, kind="ExternalOutput")

    with TileContext(nc) as tc:
        with (
            tc.tile_pool(name="const", bufs=1) as constp,
            tc.tile_pool(name="acts", bufs=1) as actp,
            tc.tile_pool(name="wpool", bufs=3) as wp,
            tc.tile_pool(name="psum", bufs=4, space="PSUM") as pp,
            tc.tile_pool(name="outp", bufs=1) as op,
        ):
            # bias/wout ride the gpsimd SWDGE queue: small rows would clog the
            # HW queues that the x pairs and weights need at startup
            bias_t = constp.tile([128, L * NT], f32, tag="bias")
            nc.gpsimd.dma_start(bias_t[:], biasin[:])
            wout_t = constp.tile([128, wout_w], dt_w, tag="wout")
            nc.gpsimd.dma_start(wout_t[:], woutin[:])

            plane0 = actp.tile([128, NT * BC], dt_w, tag="plane0")
            plane1 = actp.tile([128, NT * BC], dt_w, tag="plane1")
            planes = [plane0, plane1]
            # layer-1 input: 16 chunk-pair tiles on the scalar HWDGE queue so
            # the first matmuls start as soon as pair 0 lands (the weight
            # stream has the sync queue to itself).
            xtiles = [
                actp.tile([128, 2 * BC], dt_w, tag=f"xt{dd}", name=f"xt{dd}")
                for dd in range(ND)
            ]
            # first weight tile ahead of the x pairs sharing the sync queue
            wt00 = wp.tile([128, H], dt_w, tag="wt", name="wt00")
            nc.sync.dma_start(wt00[:], win[0, 0])
            for dd in range(ND):
                eng = nc.scalar if dd % 2 == 0 else nc.sync
                eng.dma_start(xtiles[dd][:], xin[dd])

            # layer-3 output as per-tile tiles so the final layer's chunk-c
            # matmuls start as soon as Sign t=c lands (not after all 32)
            dtiles = [
                actp.tile([128, BC], dt_w, tag=f"dt{t}", name=f"dt{t}")
                for t in range(NT)
            ]

            cur = 0
            for l in range(L):
                src, dst = planes[cur], planes[1 - cur]
                src3 = src[:].rearrange("p (c b) -> p c b", c=NT)
                for t in range(NT):
                    if l == 0 and t == 0:
                        wt = wt00
                    else:
                        wt = wp.tile([128, H], dt_w, tag="wt")
                        nc.sync.dma_start(wt[:], win[l, t])
                    ps = pp.tile([128, BC], f32, tag="ps")
                    if mode == "fp8dr":
                        w3 = wt[:].rearrange("p (d j m) -> p d j m", d=ND, j=2)
                        for d in range(ND):
                            lhsT = w3[:, d]
                            if l == 0:
                                x3 = xtiles[d][:].rearrange("p (j b) -> p j b", j=2)
                                a0 = x3[:, :, 0:HALF]
                                a1 = x3[:, :, HALF:BC]
                            else:
                                a0 = src3[:, 2 * d : 2 * d + 2, 0:HALF]
                                a1 = src3[:, 2 * d : 2 * d + 2, HALF:BC]
                            nc.tensor.matmul(
                                ps[:, 0:HALF], lhsT, a0,
                                start=(d == 0), stop=(d == ND - 1),
                                perf_mode=mybir.MatmulPerfMode.DoubleRow,
                            )
                            nc.tensor.matmul(
                                ps[:, HALF:BC], lhsT, a1,
                                start=(d == 0), stop=(d == ND - 1),
                                perf_mode=mybir.MatmulPerfMode.DoubleRow,
                            )
                    else:
                        for c in range(NT):
                            lhsT = wt[:, c * 128 : (c + 1) * 128]
                            if l == 0:
                                xs = xtiles[c // 2][:, (c % 2) * BC : (c % 2 + 1) * BC]
                                a0 = xs[:, 0:HALF]
                                a1 = xs[:, HALF:BC]
                            else:
                                a0 = src[:, c * BC : c * BC + HALF]
                                a1 = src[:, c * BC + HALF : (c + 1) * BC]
                            nc.tensor.matmul(
                                ps[:, 0:HALF], lhsT, a0,
                                start=(c == 0), stop=(c == NT - 1),
                            )
                            nc.tensor.matmul(
                                ps[:, HALF:BC], lhsT, a1,
                                start=(c == 0), stop=(c == NT - 1),
                            )
                    sign_dst = (
                        dtiles[t][:]
                        if l == L - 1
                        else dst[:, t * BC : (t + 1) * BC]
                    )
                    nc.scalar.sign(
                        sign_dst,
                        ps[:],
                        bias=bias_t[:, l * NT + t : l * NT + t + 1],
                    )
                cur = 1 - cur
            # final 10-channel layer: 4-way column tiling — chunk c runs in
            # column group c%4 (concurrent in the PE array), partial sums land
            # at PSUM partitions 32g..32g+9 and are reduced on DVE.
            psf = pp.tile([128, BC], f32, tag="ps", name="psf")
            for c in range(NT):
                g = c % 4
                lhsT = wout_t[:, c * NCOUT : (c + 1) * NCOUT]
                a0 = dtiles[c][:, 0:HALF]
                a1 = dtiles[c][:, HALF:BC]
                nc.tensor.matmul(
                    psf[32 * g : 32 * g + NCOUT, 0:HALF], lhsT, a0,
                    start=(c < 4), stop=(c >= NT - 4), tile_position=(0, 32 * g),
                )
                nc.tensor.matmul(
                    psf[32 * g : 32 * g + NCOUT, HALF:BC], lhsT, a1,
                    start=(c < 4), stop=(c >= NT - 4), tile_position=(0, 32 * g),
                )
            s0 = op.tile([NCOUT, BC], f32, tag="s0")
            s1 = op.tile([NCOUT, BC], f32, tag="s1")
            nc.scalar.copy(s0[:], psf[0:NCOUT, :])
            nc.vector.tensor_add(s1[:], s0[:], psf[32 : 32 + NCOUT, :])
            nc.vector.tensor_add(s0[:], s1[:], psf[64 : 64 + NCOUT, :])
            out_t = op.tile([NCOUT, BC], f32, tag="out")
            nc.vector.tensor_add(out_t[:], s0[:], psf[96 : 96 + NCOUT, :])
            nc.sync.dma_start(outd[:], out_t[:])

    _split_multi_waits(nc)
    _BUILD_CACHE[mode] = nc
    return nc


def _thresholds(bn_gamma, bn_beta, bn_mean, bn_var):
    """Per-channel even-integer threshold T with sign(BN(y)) = +1 <=> y >= T,
    mirroring the reference's fp32 arithmetic. gamma>0 so BN is increasing."""
    arg = (bn_var.astype(np.float32) + BN_EPS).astype(np.float32)  # fp32 add as in ref
    rs = (1.0 / np.sqrt(arg.astype(np.float64))).astype(np.float32)
    y = np.arange(-H, H + 1, 2, dtype=np.float32)[:, None]  # [4097, 1]
    T = np.empty((L, H), np.float32)
    for l in range(L):
        z = ((y - bn_mean[l]) * rs[l]) * bn_gamma[l] + bn_beta[l]
        nz = z >= 0
        first = nz.argmax(axis=0)
        anyt = nz.any(axis=0)
        T[l] = np.where(anyt, -H + 2.0 * first, H + 2.0)
    return T


def kernel(x, W, Wout, bn_gamma, bn_beta, bn_mean, bn_var, tn_w, tn_b, tn_m, tn_v):
    global LAST_EXEC_NS
    from concourse.bass_utils import run_bass_kernel_spmd

    x = np.asarray(x, dtype=np.float32)
    W = np.asarray(W, dtype=np.float32)
    Wout = np.asarray(Wout, dtype=np.float32)
    bn_gamma = np.asarray(bn_gamma, dtype=np.float32)
    bn_beta = np.asarray(bn_beta, dtype=np.float32)
    bn_mean = np.asarray(bn_mean, dtype=np.float32)
    bn_var = np.asarray(bn_var, dtype=np.float32)

    mode = MODE
    np_dt = ml_dtypes.float8_e4m3 if mode == "fp8dr" else ml_dtypes.bfloat16

    # --- host prep: binarize + lay out ---
    xb = np.where(x.reshape(B, H) >= np.float32(0.5), 1.0, -1.0).astype(np_dt)
    xb = np.ascontiguousarray(xb.T)  # [H, B] feature-major

    Ws = np.where(W >= 0, 1.0, -1.0).astype(np_dt)  # [L, O, H]
    if mode == "fp8dr":
        # w_dev[l, t, k, d*256 + j*128 + m] = Ws[l, t*128+m, (2d+j)*128+k]
        w_dev = np.ascontiguousarray(
            Ws.reshape(L, NT, 128, ND, 2, 128)
            .transpose(0, 1, 5, 3, 4, 2)
            .reshape(L, NT, 128, H)
        )
    else:
        # w_dev[l, t, k, c*128+m] = Ws[l, t*128+m, c*128+k]
        w_dev = np.ascontiguousarray(
            Ws.reshape(L, NT, 128, NT, 128)
            .transpose(0, 1, 4, 3, 2)
            .reshape(L, NT, 128, H)
        )

    T = _thresholds(bn_gamma, bn_beta, bn_mean, bn_var)
    # bias[p, l*NT+t] = 1 - T[l, t*128+p]
    bias_host = np.ascontiguousarray(
        (np.float32(1.0) - T).reshape(L, NT, 128).transpose(2, 0, 1).reshape(128, L * NT)
    )

    WoS = np.where(Wout >= 0, 1.0, -1.0).astype(np_dt)  # [10, H]
    # wout[k, c*10+j] = WoS[j, c*128+k]
    wout_host = np.ascontiguousarray(
        WoS.reshape(NCOUT, NT, 128).transpose(2, 1, 0).reshape(128, NT * NCOUT)
    )

    nc = _build(mode)
    in_maps = []
    for core in range(N_CORES):
        sl = slice(core * BC, (core + 1) * BC)
        # pair-major: xc[d, p, j*BC+b] = xb[(2d+j)*128 + p, b]
        xc = np.ascontiguousarray(
            xb[:, sl].reshape(ND, 2, 128, BC).transpose(0, 2, 1, 3).reshape(
                ND, 128, 2 * BC
            )
        )
        in_maps.append(
            {"x": xc, "w": w_dev, "bias": bias_host, "wout": wout_host}
        )

    kwargs = {}
    if TRACE:
        kwargs = {"trace": True, "tmpdir": TRACE_DIR}
    res = run_bass_kernel_spmd(nc, in_maps, list(range(N_CORES)), **kwargs)
    LAST_EXEC_NS = res.exec_time_ns

    out_int = np.concatenate(
        [np.asarray(res.results[c]["out"], dtype=np.float32).T for c in range(N_CORES)],
        axis=0,
    )  # [B, 10] exact even integers

    rs_t = np.float32(1.0 / np.sqrt(np.float64(np.float32(tn_v) + TN_EPS)))
    out = ((out_int - np.float32(tn_m)) * rs_t) * np.float32(tn_w) + np.float32(tn_b)
    return out.astype(np.float32)


# revision 36
# speedup vs baseline: 1.0027x; 1.0027x over previous
"""Binarized 3-layer MLP (B=8192, H=4096) on 8 Trainium2 NeuronCores.

Strategy: data-parallel over batch (1024 rows/core), weights replicated.
All matmul operands are exactly +-1, so the GEMMs are exact in bf16/fp8
(products +-1, fp32 PSUM accumulation of <=4096 terms). BatchNorm+binarize
folds into an integer threshold per output channel: the GEMM output y is an
even integer in [-4096, 4096] and gamma*rsqrt(var+eps) > 0, so
  sign(BN(y)) = +1  <=>  y >= T_o
for an even-integer threshold T_o computed on the host. On-device this is a
single ScalarE Sign activation with per-partition bias 1 - T_o (y + 1 - T_o
is an odd integer, so no 0-boundary ambiguity).

Layout is feature-major throughout: activations live in SBUF as
[128 partitions (h within chunk), 32 chunks x 1024 batch]; each layer is
32 o-tiles x (32 h-chunks x 2 batch-halves) accumulating matmuls
(lhsT = W^T tile [128h, 128o], rhs = act [128h, 512b]) followed by one
Sign over the [128, 1024] PSUM tile, written directly into the other
activation plane. No transposes, no collectives.
"""

import numpy as np
import ml_dtypes

N_CORES = 8
B, H, L, NCOUT = 8192, 4096, 3, 10
BC = B // N_CORES          # batch per core
NT = H // 128              # 32 tiles of 128 along any H axis
BN_EPS = np.float32(1e-5)
TN_EPS = np.float32(1e-4)
HALF = BC // 2             # 512: one PSUM bank of fp32 per matmul

TRACE = False              # test harness may flip this for NTFF profiling
TRACE_DIR = None
LAST_EXEC_NS = None
MODE = "fp8dr"             # "fp8dr" (fp8 + DoubleRow, 2x contraction/MM) or "bf16"
ND = H // 256              # 16 double-row chunks of 256 along contraction

_BUILD_CACHE = {}


def _split_multi_waits(nc):
    """walrus' CoreV3 codegen rejects instructions carrying more than one
    semaphore wait. Hoist all-but-one wait of any multi-wait instruction
    into standalone NoOps (same engine, placed immediately before)."""
    import bass_rust
    import concourse.mybir as mybir

    n = 0
    for f in nc.m.functions:
        for blk in f.blocks:
            out = []
            changed = False
            for inst in blk.instructions:
                si = inst.sync_info
                if si is not None and si.on_wait and len(si.on_wait) > 1:
                    waits = list(si.on_wait)
                    for w in waits[:-1]:
                        n += 1
                        nop = mybir.InstNoOp(name=f"waitsplit_{n}", ins=[], outs=[])
                        nop.engine = inst.engine
                        nop.sync_info = bass_rust.SyncInfo(on_wait=[w], on_update=[])
                        out.append(nop)
                    inst.sync_info = bass_rust.SyncInfo(
                        on_wait=[waits[-1]], on_update=list(si.on_update or [])
                    )
                    changed = True
                out.append(inst)
            if changed:
                blk.instructions = out
    return nc


def _build(mode):
    if mode in _BUILD_CACHE:
        return _BUILD_CACHE[mode]

    import concourse.bass as bass
    import concourse.mybir as mybir
    from concourse.tile import TileContext

    dt_w = mybir.dt.float8e4 if mode == "fp8dr" else mybir.dt.bfloat16
    f32 = mybir.dt.float32

    wout_w = NT * NCOUT
    nc = bass.Bass()
    xin = nc.dram_tensor("x", [ND, 128, 2 * BC], dt_w, kind="ExternalInput")
    win = nc.dram_tensor("w", [L, NT, 128, H], dt_w, kind="ExternalInput")
    biasin = nc.dram_tensor("bias", [128, L * NT], f32, kind="ExternalInput")
    woutin = nc.dram_tensor("wout", [128, wout_w], dt_w, kind="ExternalInput")
    outd = nc.dram_tensor("out", [NCOUT, BC], f32, kind="ExternalOutput")

    with TileContext(nc) as tc:
        with (
            tc.tile_pool(name="const", bufs=1) as constp,
            tc.tile_pool(name="acts", bufs=1) as actp,
            tc.tile_pool(name="wpool", bufs=3) as wp,
            tc.tile_pool(name="psum", bufs=4, space="PSUM") as pp,
            tc.tile_pool(name="outp", bufs=1) as op,
        ):
            # bias/wout ride the gpsimd SWDGE queue: small rows would clog the
            # HW queues that the x pairs and weights need at startup
            bias_t = constp.tile([128, L * NT], f32, tag="bias")
            nc.gpsimd.dma_start(bias_t[:], biasin[:])
            wout_t = constp.tile([128, wout_w], dt_w, tag="wout")
            nc.gpsimd.dma_start(wout_t[:], woutin[:])

            plane0 = actp.tile([128, NT * BC], dt_w, tag="plane0")
            plane1 = actp.tile([128, NT * BC], dt_w, tag="plane1")
            planes = [plane0, plane1]
            # layer-1 input: 16 chunk-pair tiles on the scalar HWDGE queue so
            # the first matmuls start as soon as pair 0 lands (the weight
            # stream has the sync queue to itself).
            xtiles = [
                actp.tile([128, 2 * BC], dt_w, tag=f"xt{dd}", name=f"xt{dd}")
                for dd in range(ND)
            ]
            # first weight tile ahead of the x pairs sharing the sync queue
            wt00 = wp.tile([128, H], dt_w, tag="wt", name="wt00")
            nc.sync.dma_start(wt00[:], win[0, 0])
            for dd in range(ND):
                eng = nc.scalar if dd % 2 == 0 else nc.sync
                eng.dma_start(xtiles[dd][:], xin[dd])

            cur = 0
            for l in range(L):
                src, dst = planes[cur], planes[1 - cur]
                src3 = src[:].rearrange("p (c b) -> p c b", c=NT)
                for t in range(NT):
                    if l == 0 and t == 0:
                        wt = wt00
                    else:
                        wt = wp.tile([128, H], dt_w, tag="wt")
                        nc.sync.dma_start(wt[:], win[l, t])
                    ps = pp.tile([128, BC], f32, tag="ps")
                    if mode == "fp8dr":
                        w3 = wt[:].rearrange("p (d j m) -> p d j m", d=ND, j=2)
                        for d in range(ND):
                            lhsT = w3[:, d]
                            if l == 0:
                                x3 = xtiles[d][:].rearrange("p (j b) -> p j b", j=2)
                                a0 = x3[:, :, 0:HALF]
                                a1 = x3[:, :, HALF:BC]
                            else:
                                a0 = src3[:, 2 * d : 2 * d + 2, 0:HALF]
                                a1 = src3[:, 2 * d : 2 * d + 2, HALF:BC]
                            nc.tensor.matmul(
                                ps[:, 0:HALF], lhsT, a0,
                                start=(d == 0), stop=(d == ND - 1),
                                perf_mode=mybir.MatmulPerfMode.DoubleRow,
                            )
                            nc.tensor.matmul(
                                ps[:, HALF:BC], lhsT, a1,
                                start=(d == 0), stop=(d == ND - 1),
                                perf_mode=mybir.MatmulPerfMode.DoubleRow,
                            )
                    else:
                        for c in range(NT):
                            lhsT = wt[:, c * 128 : (c + 1) * 128]
                            if l == 0:
                                xs = xtiles[c // 2][:, (c % 2) * BC : (c % 2 + 1) * BC]
                                a0 = xs[:, 0:HALF]
                                a1 = xs[:, HALF:BC]
                            else:
                                a0 = src[:, c * BC : c * BC + HALF]
                                a1 = src[:, c * BC + HALF : (c + 1) * BC]
                            nc.tensor.matmul(
                                ps[:, 0:HALF], lhsT, a0,
                                start=(c == 0), stop=(c == NT - 1),
                            )
                            nc.tensor.matmul(
                                ps[:, HALF:BC], lhsT, a1,
                                start=(c == 0), stop=(c == NT - 1),
                            )
                    nc.scalar.sign(
                        dst[:, t * BC : (t + 1) * BC],
                        ps[:],
                        bias=bias_t[:, l * NT + t : l * NT + t + 1],
                    )
                cur = 1 - cur

            src = planes[cur]
            # final 10-channel layer: 4-way column tiling — chunk c runs in
            # column group c%4 (concurrent in the PE array), partial sums land
            # at PSUM partitions 32g..32g+9 and are reduced on DVE.
            psf = pp.tile([128, BC], f32, tag="ps", name="psf")
            for c in range(NT):
                g = c % 4
                lhsT = wout_t[:, c * NCOUT : (c + 1) * NCOUT]
                a0 = src[:, c * BC : c * BC + HALF]
                a1 = src[:, c * BC + HALF : (c + 1) * BC]
                nc.tensor.matmul(
                    psf[32 * g : 32 * g + NCOUT, 0:HALF], lhsT, a0,
                    start=(c < 4), stop=(c >= NT - 4), tile_position=(0, 32 * g),
                )
                nc.tensor.matmul(
                    psf[32 * g : 32 * g + NCOUT, HALF:BC], lhsT, a1,
                    start=(c < 4), stop=(c >= NT - 4), tile_position=(0, 32 * g),
                )
            s0 = op.tile([NCOUT, BC], f32, tag="s0")
            s1 = op.tile([NCOUT, BC], f32, tag="s1")
            nc.scalar.copy(s0[:], psf[0:NCOUT, :])
            nc.vector.tensor_add(s1[:], s0[:], psf[32 : 32 + NCOUT, :])
            nc.vector.tensor_add(s0[:], s1[:], psf[64 : 64 + NCOUT, :])
            out_t = op.tile([NCOUT, BC], f32, tag="out")
            nc.vector.tensor_add(out_t[:], s0[:], psf[96 : 96 + NCOUT, :])
            nc.sync.dma_start(outd[:], out_t[:])

    _split_multi_waits(nc)
    _BUILD_CACHE[mode] = nc
    return nc


def _thresholds(bn_gamma, bn_beta, bn_mean, bn_var):
    """Per-channel even-integer threshold T with sign(BN(y)) = +1 <=> y >= T,
    mirroring the reference's fp32 arithmetic. gamma>0 so BN is increasing."""
    arg = (bn_var.astype(np.float32) + BN_EPS).astype(np.float32)  # fp32 add as in ref
    rs = (1.0 / np.sqrt(arg.astype(np.float64))).astype(np.float32)
    y = np.arange(-H, H + 1, 2, dtype=np.float32)[:, None]  # [4097, 1]
    T = np.empty((L, H), np.float32)
    for l in range(L):
        z = ((y - bn_mean[l]) * rs[l]) * bn_gamma[l] + bn_beta[l]
        nz = z >= 0
        first = nz.argmax(axis=0)
        anyt = nz.any(axis=0)
        T[l] = np.where(anyt, -H + 2.0 * first, H + 2.0)
    return T


def kernel(x, W, Wout, bn_gamma, bn_beta, bn_mean, bn_var, tn_w, tn_b, tn_m, tn_v):
    global LAST_EXEC_NS
    from concourse.bass_utils import run_bass_kernel_spmd

    x = np.asarray(x, dtype=np.float32)
    W = np.asarray(W, dtype=np.float32)
    Wout = np.asarray(Wout, dtype=np.float32)
    bn_gamma = np.asarray(bn_gamma, dtype=np.float32)
    bn_beta = np.asarray(bn_beta, dtype=np.float32)
    bn_mean = np.asarray(bn_mean, dtype=np.float32)
    bn_var = np.asarray(bn_var, dtype=np.float32)

    mode = MODE
    np_dt = ml_dtypes.float8_e4m3 if mode == "fp8dr" else ml_dtypes.bfloat16

    # --- host prep: binarize + lay out ---
    xb = np.where(x.reshape(B, H) >= np.float32(0.5), 1.0, -1.0).astype(np_dt)
    xb = np.ascontiguousarray(xb.T)  # [H, B] feature-major

    Ws = np.where(W >= 0, 1.0, -1.0).astype(np_dt)  # [L, O, H]
    if mode == "fp8dr":
        # w_dev[l, t, k, d*256 + j*128 + m] = Ws[l, t*128+m, (2d+j)*128+k]
        w_dev = np.ascontiguousarray(
            Ws.reshape(L, NT, 128, ND, 2, 128)
            .transpose(0, 1, 5, 3, 4, 2)
            .reshape(L, NT, 128, H)
        )
    else:
        # w_dev[l, t, k, c*128+m] = Ws[l, t*128+m, c*128+k]
        w_dev = np.ascontiguousarray(
            Ws.reshape(L, NT, 128, NT, 128)
            .transpose(0, 1, 4, 3, 2)
            .reshape(L, NT, 128, H)
        )

    T = _thresholds(bn_gamma, bn_beta, bn_mean, bn_var)
    # bias[p, l*NT+t] = 1 - T[l, t*128+p]
    bias_host = np.ascontiguousarray(
        (np.float32(1.0) - T).reshape(L, NT, 128).transpose(2, 0, 1).reshape(128, L * NT)
    )

    WoS = np.where(Wout >= 0, 1.0, -1.0).astype(np_dt)  # [10, H]
    # wout[k, c*10+j] = WoS[j, c*128+k]
    wout_host = np.ascontiguousarray(
        WoS.reshape(NCOUT, NT, 128).transpose(2, 1, 0).reshape(128, NT * NCOUT)
    )

    nc = _build(mode)
    in_maps = []
    for core in range(N_CORES):
        sl = slice(core * BC, (core + 1) * BC)
        # pair-major: xc[d, p, j*BC+b] = xb[(2d+j)*128 + p, b]
        xc = np.ascontiguousarray(
            xb[:, sl].reshape(ND, 2, 128, BC).transpose(0, 2, 1, 3).reshape(
                ND, 128, 2 * BC
            )
        )
        in_maps.append(
            {"x": xc, "w": w_dev, "bias": bias_host, "wout": wout_host}
        )

    kwargs = {}
    if TRACE:
        kwargs = {"trace": True, "tmpdir": TRACE_DIR}
    res = run_bass_kernel_spmd(nc, in_maps, list(range(N_CORES)), **kwargs)
    LAST_EXEC_NS = res.exec_time_ns

    out_int = np.concatenate(
        [np.asarray(res.results[c]["out"], dtype=np.float32).T for c in range(N_CORES)],
        axis=0,
    )  # [B, 10] exact even integers

    rs_t = np.float32(1.0 / np.sqrt(np.float64(np.float32(tn_v) + TN_EPS)))
    out = ((out_int - np.float32(tn_m)) * rs_t) * np.float32(tn_w) + np.float32(tn_b)
    return out.astype(np.float32)
